# Initial kernel scaffold
#
"""AttnCutLoss Trainium2 kernel.

Reference math (B=4096 rows, S=4096 positions, f1 metric, tau=0.95):
    tp    = cumsum(labels, axis=1)
    prec  = tp / k ;  rec = tp / total   (total = row sum of labels)
    r     = 2*prec*rec/(prec+rec)  ==  2*tp / (k + total)     [exact algebraic simplification,
                                                               incl. the tp==0 / total==0 guards]
    q     = exp(r/tau); norm = sum_j q;  w = 1/norm
    loss  = -sum(log(output)*w)/B  =  -(1/B) * sum_rows [ (sum_j log(output)) / norm ]

Per-core device pipeline (data-parallel over rows, 512 rows/core, 4 groups of 128):
    scan (DVE tensor_tensor_scan)  : tp = cumsum(labels)             [one op per group]
    PE transpose + K=3 fp16 matmul : d  = total + k_hi + k_lo        [exact: all integers < 2048/4096]
    DVE scalar_tensor_tensor       : r  = tp / d
    ACT Exp(scale=2/tau), accum_out: norm_row
    ACT Ln, accum_out              : logsum_row
Host: loss = -(sum over rows logsum/norm)/B.
"""

import numpy as np
import ml_dtypes

B = 4096
S = 4096
TAU = 0.95
NCORES = 8
RPC = B // NCORES          # rows per core = 512
G = RPC // 128             # row groups per core = 4

_PROGRAM_CACHE = {}


def _build_program(use_scan_fp16: bool):
    import concourse.bass as bass
    import concourse.tile as tile
    import concourse.mybir as mybir
    from contextlib import ExitStack

    dt = mybir.dt
    alu = mybir.AluOpType
    act = mybir.ActivationFunctionType

    nc = bass.Bass()
    outv = nc.dram_tensor("outv", [RPC, S], dt.float32, kind="ExternalInput")
    lab16 = nc.dram_tensor("lab16", [RPC, S], dt.bfloat16, kind="ExternalInput")
    # denrhs rows: [ones, k_hi, k_lo] with k = j+1 = k_hi + k_lo, both fp16-exact
    denrhs = nc.dram_tensor("denrhs", [3, S], dt.float16, kind="ExternalInput")
    ones2 = nc.dram_tensor("ones2", [2, 128], dt.float16, kind="ExternalInput")
    ident = nc.dram_tensor("ident", [128, 128], dt.float32, kind="ExternalInput")
    norms = nc.dram_tensor("norms", [128, G], dt.float32, kind="ExternalOutput")
    logsums = nc.dram_tensor("logsums", [128, G], dt.float32, kind="ExternalOutput")

    tp_dt = dt.float16 if use_scan_fp16 else dt.float32

    with ExitStack() as ctx:
        tc = ctx.enter_context(tile.TileContext(nc))
        consts = ctx.enter_context(tc.tile_pool(name="consts", bufs=1))
        labp = ctx.enter_context(tc.tile_pool(name="labp", bufs=2))
        outp = ctx.enter_context(tc.tile_pool(name="outp", bufs=2))
        tpp = ctx.enter_context(tc.tile_pool(name="tpp", bufs=2))
        rp = ctx.enter_context(tc.tile_pool(name="rp", bufs=2))
        dump = ctx.enter_context(tc.tile_pool(name="dump", bufs=1))
        accp = ctx.enter_context(tc.tile_pool(name="accp", bufs=1))
        dlp = ctx.enter_context(tc.tile_pool(name="dlp", bufs=4))
        dpsum = ctx.enter_context(tc.tile_pool(name="dpsum", bufs=3, space="PSUM"))
        tpsum = ctx.enter_context(tc.tile_pool(name="tpsum", bufs=2, space="PSUM"))

        denrhs_sb = consts.tile([3, S], dt.float16)
        nc.sync.dma_start(denrhs_sb[:, :], denrhs[:, :])
        ident_sb = consts.tile([128, 128], dt.float32)
        nc.sync.dma_start(ident_sb[:, :], ident[:, :])

        norms_sb = accp.tile([128, G], dt.float32)
        logsums_sb = accp.tile([128, G], dt.float32)
        qdump = dump.tile([128, S], dt.bfloat16)
        ldump = dump.tile([128, S], dt.bfloat16)

        CH = 1024  # psum chunk (2 banks)

        for g in range(G):
            lab_t = labp.tile([128, S], dt.bfloat16)
            nc.sync.dma_start(lab_t[:, :], lab16[g * 128:(g + 1) * 128, :])
            out_t = outp.tile([128, S], dt.float32)
            nc.sync.dma_start(out_t[:, :], outv[g * 128:(g + 1) * 128, :])

            # tp = cumsum(labels) along free dim; fp32 state; exact (integers)
            tp_t = tpp.tile([128, S], tp_dt)
            nc.vector.tensor_tensor_scan(
                tp_t[:, :], lab_t[:, :], lab_t[:, :], 0.0, alu.add, alu.bypass
            )

            # total per row -> [1,128] psum row via PE transpose
            trow = tpsum.tile([1, 128], tp_dt)
            nc.tensor.transpose(trow[:, :], tp_t[:, S - 1:S], ident_sb[:, :])

            # denominator stationary: [T; ones; ones] fp16
            denlhs = dlp.tile([3, 128], dt.float16)
            nc.sync.dma_start(denlhs[1:3, :], ones2[:, :])
            nc.scalar.copy(denlhs[0:1, :], trow[:, :])

            r_t = rp.tile([128, S], dt.float32)
            for h in range(S // CH):
                d_ps = dpsum.tile([128, CH], dt.float32)
                for n in range(CH // 512):
                    lo = h * CH + n * 512
                    nc.tensor.matmul(
                        d_ps[:, n * 512:(n + 1) * 512],
                        denlhs[:, :],
                        denrhs_sb[:, lo:lo + 512],
                        start=True,
                        stop=True,
                    )
                # r = (tp * 1) / d
                nc.vector.scalar_tensor_tensor(
                    r_t[:, h * CH:(h + 1) * CH],
                    tp_t[:, h * CH:(h + 1) * CH],
                    1.0,
                    d_ps[:, :],
                    alu.mult,
                    alu.divide,
                )

            nc.scalar.activation(
                qdump[:, :], r_t[:, :], act.Exp,
                scale=2.0 / TAU,
                accum_out=norms_sb[:, g:g + 1],
            )
            nc.scalar.activation(
                ldump[:, :], out_t[:, :], act.Ln,
                accum_out=logsums_sb[:, g:g + 1],
            )

        nc.sync.dma_start(norms[:, :], norms_sb[:, :])
        nc.sync.dma_start(logsums[:, :], logsums_sb[:, :])

    return nc


def _make_consts():
    j = np.arange(S, dtype=np.int64)
    k = j + 1
    k_hi = (j // 64) * 64
    k_lo = k - k_hi
    denrhs = np.stack([
        np.ones(S, dtype=np.float64), k_hi.astype(np.float64), k_lo.astype(np.float64)
    ]).astype(np.float16)
    assert np.all(denrhs[1].astype(np.int64) == k_hi)
    assert np.all(denrhs[2].astype(np.int64) == k_lo)
    ones2 = np.ones((2, 128), dtype=np.float16)
    ident = np.eye(128, dtype=np.float32)
    return denrhs, ones2, ident


def _run(output, labels, trace=False):
    from concourse.bass_utils import run_bass_kernel_spmd

    output = np.asarray(output)
    labels = np.asarray(labels)
    assert output.shape == (B, S, 1) and labels.shape == (B, S)

    outv_full = np.ascontiguousarray(output.reshape(B, S).astype(np.float32, copy=False))
    lab_full = labels.astype(ml_dtypes.bfloat16)  # 0.0/1.0 exact
    # fp16 tp would overflow above 2047; the data is ~20% dense so max total ~900.
    use_fp16 = False

    key = (use_fp16,)
    if key not in _PROGRAM_CACHE:
        _PROGRAM_CACHE[key] = _build_program(use_fp16)
    nc = _PROGRAM_CACHE[key]

    denrhs, ones2, ident = _make_consts()
    in_maps = []
    for c in range(NCORES):
        sl = slice(c * RPC, (c + 1) * RPC)
        in_maps.append({
            "outv": np.ascontiguousarray(outv_full[sl]),
            "lab16": np.ascontiguousarray(lab_full[sl]),
            "denrhs": denrhs,
            "ones2": ones2,
            "ident": ident,
        })

    res = run_bass_kernel_spmd(nc, in_maps, core_ids=list(range(NCORES)), trace=trace)

    total = 0.0
    for c in range(NCORES):
        norms = np.asarray(res.results[c]["norms"], dtype=np.float64)
        logsums = np.asarray(res.results[c]["logsums"], dtype=np.float64)
        total += float(np.sum(logsums / norms))
    loss = np.float32(-total / B)
    return loss, res


def kernel(output, labels):
    loss, _ = _run(output, labels, trace=False)
    return loss


# revision 14
# speedup vs baseline: 1.0799x; 1.0799x over previous
"""AttnCutLoss Trainium2 kernel.

Reference math (B=4096 rows, S=4096 positions, f1 metric, tau=0.95):
    tp    = cumsum(labels, axis=1)
    prec  = tp / k ;  rec = tp / total   (total = row sum of labels)
    r     = 2*prec*rec/(prec+rec)  ==  2*tp / (k + total)     [exact algebraic simplification,
                                                               incl. the tp==0 / total==0 guards]
    q     = exp(r/tau); norm = sum_j q;  w = 1/norm
    loss  = -sum(log(output)*w)/B  =  -(1/B) * sum_rows [ (sum_j log(output)) / norm ]

Per-core device pipeline (pure data parallel: 512 rows/core, 4 groups of 128 rows).
Phase A per group (emitted for all groups first so VectorE runs the scans
back-to-back): DMA labels(bf16)+output(f32); DVE tensor_tensor_scan -> tp =
cumsum(labels) in one op; PE transpose of tp[:,-1] + ACT cast -> T as an fp16
[1,128] stationary sliver. Phase B per group, in [128,1024] chunks: d = T +
k_hi + k_lo via two accumulating PE matmuls (K=2 const + K=1 T-sliver; all
operands fp16-exact integers, fp32 psum); DVE reciprocal_approx_fast(d) (~51
ULP, amply accurate since the final loss only sees ~1e-7); DVE
scalar_tensor_tensor r = tp*inv; ACT Exp(scale=2/tau) with accum_out giving
the row normalizer for free; ACT Ln(output) with accum_out giving row log-sums.
(divide/tt-divide is not encodable on this DVE; a full PE-matmul cumsum was
tried and is slower — ~100ns fixed cost per matmul x 400+ matmuls.)
Host: loss = -(sum over rows logsum_row/norm_row)/B.
"""

import numpy as np
import ml_dtypes

B = 4096
S = 4096
TAU = 0.95
NCORES = 8
RPC = B // NCORES          # rows per core = 512
G = RPC // 128             # row groups per core = 4

_PROGRAM_CACHE = {}
USE_FP16_SCAN = False


def _build_program(use_scan_fp16: bool, repeats: int = 1):
    import concourse.bass as bass
    import concourse.tile as tile
    import concourse.mybir as mybir
    from concourse import bacc
    from contextlib import ExitStack

    dt = mybir.dt
    alu = mybir.AluOpType
    act = mybir.ActivationFunctionType

    nc = bacc.Bacc("TRN2")
    outv = nc.dram_tensor("outv", [RPC, S], dt.float32, kind="ExternalInput")
    lab16 = nc.dram_tensor("lab16", [RPC, S], dt.bfloat16, kind="ExternalInput")
    # denk rows: [k_hi, k_lo] with k = j+1 = k_hi + k_lo, both fp16-exact
    denk = nc.dram_tensor("denk", [2, S], dt.float16, kind="ExternalInput")
    denones = nc.dram_tensor("denones", [1, S], dt.float16, kind="ExternalInput")
    ones2 = nc.dram_tensor("ones2", [2, 128], dt.float16, kind="ExternalInput")
    ident = nc.dram_tensor("ident", [128, 128], dt.float32, kind="ExternalInput")
    identh = nc.dram_tensor("identh", [128, 128], dt.float16, kind="ExternalInput")
    norms = nc.dram_tensor("norms", [128, G * 4], dt.float32, kind="ExternalOutput")
    logsums = nc.dram_tensor("logsums", [128, G], dt.float32, kind="ExternalOutput")

    tp_dt = dt.float16 if use_scan_fp16 else dt.float32

    with ExitStack() as ctx:
        tc = ctx.enter_context(tile.TileContext(nc))
        consts = ctx.enter_context(tc.tile_pool(name="consts", bufs=1))
        labp = ctx.enter_context(tc.tile_pool(name="labp", bufs=4))
        outp = ctx.enter_context(tc.tile_pool(name="outp", bufs=2))
        tpp = ctx.enter_context(tc.tile_pool(name="tpp", bufs=4))
        rp = ctx.enter_context(tc.tile_pool(name="rp", bufs=3))
        dump = ctx.enter_context(tc.tile_pool(name="dump", bufs=1))
        accp = ctx.enter_context(tc.tile_pool(name="accp", bufs=1))
        dlp = ctx.enter_context(tc.tile_pool(name="dlp", bufs=4))
        invp = ctx.enter_context(tc.tile_pool(name="invp", bufs=3))
        dpsum = ctx.enter_context(tc.tile_pool(name="dpsum", bufs=3, space="PSUM"))
        tpsum = ctx.enter_context(tc.tile_pool(name="tpsum", bufs=2, space="PSUM"))

        denk_sb = consts.tile([2, S], dt.float16)
        nc.sync.dma_start(denk_sb[:, :], denk[:, :])
        denones_sb = consts.tile([1, S], dt.float16)
        nc.sync.dma_start(denones_sb[:, :], denones[:, :])
        ident_sb = consts.tile([128, 128], dt.float32)
        nc.sync.dma_start(ident_sb[:, :], ident[:, :])
        identh_sb = consts.tile([128, 128], dt.float16)
        nc.sync.dma_start(identh_sb[:, :], identh[:, :])
        ones2_sb = consts.tile([2, 128], dt.float16)
        nc.sync.dma_start(ones2_sb[:, :], ones2[:, :])

        naccs_sb = accp.tile([128, G * 4], dt.float32)
        logsums_sb = accp.tile([128, G], dt.float32)
        qdump = dump.tile([128, S], dt.bfloat16)
        ldump = dump.tile([128, S], dt.bfloat16)

        CH = 1024  # psum chunk (2 banks)

        import contextlib
        loop_cm = tc.For_i(0, repeats, 1) if repeats > 1 else contextlib.nullcontext()
        with loop_cm:
          tp_ts = []
          out_ts = []
          tcasts = []
          # Phase A: DMAs + scans back-to-back (VectorE saturated) + T-chains
          for g in range(G):
              lab_t = labp.tile([128, S], dt.bfloat16, tag="lab")
              nc.sync.dma_start(lab_t[:, :], lab16[g * 128:(g + 1) * 128, :])
              out_t = outp.tile([128, S], dt.float32, tag="outv")
              nc.sync.dma_start(out_t[:, :], outv[g * 128:(g + 1) * 128, :])
              out_ts.append(out_t)

              tp_t = tpp.tile([128, S], tp_dt, tag="tp")
              nc.vector.tensor_tensor_scan(
                  tp_t[:, :], lab_t[:, :], lab_t[:, :], 0.0, alu.add, alu.bypass
              )
              tp_ts.append(tp_t)

              trow = tpsum.tile([1, 128], tp_dt, tag="trow")
              nc.tensor.transpose(trow[:, :], tp_t[:, S - 1:S],
                                  identh_sb[:, :] if use_scan_fp16 else ident_sb[:, :])
              tcast = dlp.tile([1, 128], dt.float16, tag="tcast")
              nc.scalar.copy(tcast[:, :], trow[:, :])
              tcasts.append(tcast)

          # Phase B: per group: d matmuls + recip + multiply + activations
          for g in range(G):
              for h in range(S // CH):
                  d_ps = dpsum.tile([128, CH], dt.float32, tag="dps")
                  for n in range(CH // 512):
                      lo = h * CH + n * 512
                      nsl = slice(n * 512, (n + 1) * 512)
                      nc.tensor.matmul(
                          d_ps[:, nsl], ones2_sb[:, :], denk_sb[:, lo:lo + 512],
                          start=True, stop=False)
                      nc.tensor.matmul(
                          d_ps[:, nsl], tcasts[g][:, :], denones_sb[:, lo:lo + 512],
                          start=False, stop=True)
                  inv_t = invp.tile([128, CH], dt.float32, tag="inv")
                  nc.vector.reciprocal_approx_fast(out=inv_t[:, :], in_=d_ps[:, :])
                  r_t = rp.tile([128, CH], dt.float32, tag="r")
                  nc.vector.scalar_tensor_tensor(
                      r_t[:, :],
                      tp_ts[g][:, h * CH:(h + 1) * CH],
                      1.0,
                      inv_t[:, :],
                      alu.mult,
                      alu.mult,
                  )
                  nc.scalar.activation(
                      qdump[:, h * CH:(h + 1) * CH], r_t[:, :], act.Exp,
                      scale=2.0 / TAU,
                      accum_out=naccs_sb[:, g * (S // CH) + h:g * (S // CH) + h + 1],
                  )
              nc.scalar.activation(
                  ldump[:, :], out_ts[g][:, :], act.Ln,
                  accum_out=logsums_sb[:, g:g + 1],
              )

        nc.sync.dma_start(norms[:, :], naccs_sb[:, :])
        nc.sync.dma_start(logsums[:, :], logsums_sb[:, :])

    nc.finalize()
    return nc


def _make_consts():
    j = np.arange(S, dtype=np.int64)
    k = j + 1
    k_hi = (j // 64) * 64
    k_lo = k - k_hi
    denk = np.stack([
        k_hi.astype(np.float64), k_lo.astype(np.float64)
    ]).astype(np.float16)
    assert np.all(denk[0].astype(np.int64) == k_hi)
    assert np.all(denk[1].astype(np.int64) == k_lo)
    denones = np.ones((1, S), dtype=np.float16)
    ones2 = np.ones((2, 128), dtype=np.float16)
    ident = np.eye(128, dtype=np.float32)
    identh = np.eye(128, dtype=np.float16)
    return denk, denones, ones2, ident, identh


def _run(output, labels, trace=False):
    from concourse.bass_utils import run_bass_kernel_spmd

    output = np.asarray(output)
    labels = np.asarray(labels)
    assert output.shape == (B, S, 1) and labels.shape == (B, S)

    outv_full = np.ascontiguousarray(output.reshape(B, S).astype(np.float32, copy=False))
    lab_full = labels.astype(ml_dtypes.bfloat16)  # 0.0/1.0 exact
    # fp16 tp would overflow above 2047; the data is ~20% dense so max total ~900.
    use_fp16 = USE_FP16_SCAN and float(lab_full.astype(np.float32).sum(axis=1).max()) < 2000

    key = (use_fp16,)
    if key not in _PROGRAM_CACHE:
        _PROGRAM_CACHE[key] = _build_program(use_fp16)
    nc = _PROGRAM_CACHE[key]

    denk, denones, ones2, ident, identh = _make_consts()
    in_maps = []
    for c in range(NCORES):
        sl = slice(c * RPC, (c + 1) * RPC)
        in_maps.append({
            "outv": np.ascontiguousarray(outv_full[sl]),
            "lab16": np.ascontiguousarray(lab_full[sl]),
            "denk": denk,
            "denones": denones,
            "ones2": ones2,
            "ident": ident,
            "identh": identh,
        })

    res = run_bass_kernel_spmd(nc, in_maps, core_ids=list(range(NCORES)), trace=trace)

    total = 0.0
    for c in range(NCORES):
        norms = np.asarray(res.results[c]["norms"], dtype=np.float64)
        norms = norms.reshape(128, G, 4).sum(axis=2)
        logsums = np.asarray(res.results[c]["logsums"], dtype=np.float64)
        total += float(np.sum(logsums / norms))
    loss = np.float32(-total / B)
    return loss, res


def kernel(output, labels):
    loss, _ = _run(output, labels, trace=False)
    return loss
